# revision 2
# baseline (speedup 1.0000x reference)
"""Distributed embedding-lookup kernel for Trainium2 (8 NeuronCores).

Reference computation: out = table[inputs]  with
  inputs: [4096, 26, 2] int64 indices into a [1_000_000, 32] f32 table
  out:    [4096, 26, 2, 32] f32

Strategy (model parallel, per the sharding hint): the table is sharded
row-wise across the 8 cores (125,000 rows = 16 MB each). Each core's
shard is staged in SBUF transposed as 4 stacked chunks of 31,250 rows:
SBUF tile [128, 31250] f32 where partition p holds dim (p % 32) of chunk
(p // 32). The host routes every lookup to its owner core and chunk
(pure index arithmetic, the "dispatch" half of the All2All), and the
owner core gathers all of its ~26.6k requested rows with a single
GPSIMD InstAPGather (engine-side SBUF gather: one 16-partition group
per chunk-half, shared per-group index lists, measured ~0.5 us). The
gathered vectors land dim-major ([32, n] per chunk band) in DRAM; the
host transposes/scatters them back to batch order (the "combine" half).

The ap_gather ucode lives in an overlay Q7 library; the library load is
issued as the very first GPSIMD instruction so its fixed latency
overlaps the 16 MB/core HBM->SBUF shard load.
"""

import os

import numpy as np

import concourse.bacc as bacc
import concourse.tile as tile
from concourse import bass, library_config, mybir
from concourse.bass_utils import run_bass_kernel_spmd

P = 128
N_CORES = 8
VOCAB = 1_000_000
D = 32
SHARD = VOCAB // N_CORES      # 125,000 rows per core
NCH = 4                       # chunks per core (stacked on partition axis)
CHUNK = SHARD // NCH          # 31,250 rows per chunk (int16-addressable)
NI = 7680                     # padded gather slots per chunk (mean 6656)
S = NI // 16                  # wrapped index columns
TOTAL = 4096 * 26 * 2

_CACHE = {}
LAST_RESULTS = None


def _ensure_ntff_hook():
    """Install the axon NTFF profiling hook if missing (test-only path)."""
    import sys
    import types

    if "antenv.axon_hooks" not in sys.modules:
        mod = types.ModuleType("antenv.axon_hooks")
        store = {"hook": None}
        mod.set_axon_ntff_profile_hook = lambda h: store.update(hook=h)
        mod.get_axon_ntff_profile_hook = lambda: store["hook"]
        sys.modules["antenv.axon_hooks"] = mod
        import antenv

        antenv.axon_hooks = mod
    from antenv.axon_hooks import (
        get_axon_ntff_profile_hook,
        set_axon_ntff_profile_hook,
    )

    if get_axon_ntff_profile_hook() is None:
        from trn_agent_boot.trn_boot import _ntff_profile_via_ctypes

        set_axon_ntff_profile_hook(
            _ntff_profile_via_ctypes("/opt/axon/libaxon_pjrt.so")
        )

    import concourse.bass_utils as bu

    bu.upload_artifacts = lambda tmpdir: tmpdir


def _build():
    nc = bacc.Bacc(
        "TRN2", target_bir_lowering=False, debug=False, num_devices=N_CORES
    )
    tab = nc.dram_tensor("tab", [P, CHUNK], mybir.dt.float32, kind="ExternalInput").ap()
    idx = nc.dram_tensor("idx", [P, S], mybir.dt.int16, kind="ExternalInput").ap()
    out = nc.dram_tensor("out", [P, NI], mybir.dt.float32, kind="ExternalOutput").ap()

    with tile.TileContext(nc) as tc:
        with tc.tile_pool(name="sbuf", bufs=1) as pool:
            # Fire the overlay-library load first: its fixed latency is the
            # critical path and overlaps the HWDGE shard/idx loads below.
            nc.gpsimd.load_library(library_config.ap_gather)
            tab_sb = pool.tile([P, CHUNK], mybir.dt.float32, tag="t")
            idx_sb = pool.tile([P, S], mybir.dt.int16, tag="i")
            out_sb = pool.tile([P, NI], mybir.dt.float32, tag="o")
            nc.sync.dma_start(idx_sb[:], idx[:])
            nc.sync.dma_start(tab_sb[:], tab[:])
            nc.gpsimd.ap_gather(
                out_ap=out_sb[:],
                in_ap=tab_sb[:],
                idxs_ap=idx_sb[:],
                channels=P,
                num_elems=CHUNK,
                d=1,
                num_idxs=NI,
            )
            nc.sync.dma_start(out[:], out_sb[:])
    nc.compile()
    return nc


def kernel(inputs: np.ndarray, table: np.ndarray) -> np.ndarray:
    global LAST_RESULTS
    if "nc" not in _CACHE:
        _CACHE["nc"] = _build()
    nc = _CACHE["nc"]

    trace = bool(os.environ.get("BASS_TRACE"))
    if trace:
        _ensure_ntff_hook()

    flat = np.ascontiguousarray(inputs).reshape(-1).astype(np.int64)
    table = np.ascontiguousarray(table, dtype=np.float32)
    assert flat.shape == (TOTAL,) and table.shape == (VOCAB, D)

    owner = flat // SHARD
    local = flat % SHARD
    chunk = local // CHUNK
    pos = (local % CHUNK).astype(np.int16)

    in_maps = []
    placements = []  # per core: list of (orig_positions, n) per chunk
    for o in range(N_CORES):
        tab_np = np.ascontiguousarray(
            table[o * SHARD : (o + 1) * SHARD]
            .reshape(NCH, CHUNK, D)
            .transpose(0, 2, 1)
            .reshape(P, CHUNK)
        )
        idx_groups = np.zeros((8, NI), dtype=np.int16)
        per_chunk = []
        o_mask = owner == o
        for c in range(NCH):
            orig = np.flatnonzero(o_mask & (chunk == c))
            n = len(orig)
            assert n <= NI, f"chunk overflow: {n} > {NI}"
            idx_groups[2 * c, :n] = pos[orig]
            idx_groups[2 * c + 1, :n] = pos[orig]
            per_chunk.append((orig, n))
        # group g's token j lives at idxs[16*g + j%16, j//16]
        idx_np = np.ascontiguousarray(
            idx_groups.reshape(8, S, 16).transpose(0, 2, 1).reshape(P, S)
        )
        in_maps.append({"tab": tab_np, "idx": idx_np})
        placements.append(per_chunk)

    res = run_bass_kernel_spmd(
        nc, in_maps, core_ids=list(range(N_CORES)), trace=trace
    )
    LAST_RESULTS = res

    final = np.empty((TOTAL, D), dtype=np.float32)
    for o in range(N_CORES):
        out = res.results[o]["out"].reshape(P, NI)
        for c in range(NCH):
            orig, n = placements[o][c]
            if n:
                final[orig] = out[32 * c : 32 * (c + 1), :n].T
    return final.reshape(4096, 26, 2, D)


# revision 3
# speedup vs baseline: 1.1515x; 1.1515x over previous
"""Distributed embedding-lookup kernel for Trainium2 (8 NeuronCores).

Reference computation: out = table[inputs]  with
  inputs: [4096, 26, 2] int64 indices into a [1_000_000, 32] f32 table
  out:    [4096, 26, 2, 32] f32

Strategy (model parallel, per the sharding hint): the table is sharded
row-wise across the 8 cores (125,000 rows = 16 MB each). Each core's
shard is staged in SBUF transposed as 4 stacked chunks of 31,250 rows:
SBUF tile [128, 31250] f32 where partition p holds dim (p % 32) of chunk
(p // 32). The host routes every lookup to its owner core and chunk
(pure index arithmetic, the "dispatch" half of the All2All), and the
owner core gathers all of its ~26.6k requested rows with a single
GPSIMD InstAPGather (engine-side SBUF gather: one 16-partition group
per chunk-half, shared per-group index lists, measured ~0.5 us). The
gathered vectors land dim-major ([32, n] per chunk band) in DRAM; the
host transposes/scatters them back to batch order (the "combine" half).

The ap_gather ucode lives in an overlay Q7 library; the library load is
issued as the very first GPSIMD instruction so its fixed latency
overlaps the 16 MB/core HBM->SBUF shard load.
"""

import os

import numpy as np

import concourse.bacc as bacc
import concourse.tile as tile
from concourse import bass, library_config, mybir
from concourse.bass_utils import run_bass_kernel_spmd

P = 128
N_CORES = 8
VOCAB = 1_000_000
D = 32
SHARD = VOCAB // N_CORES      # 125,000 rows per core
NCH = 4                       # chunks per core (stacked on partition axis)
CHUNK = SHARD // NCH          # 31,250 rows per chunk (int16-addressable)
NI = 7680                     # padded gather slots per chunk (mean 6656)
S = NI // 16                  # wrapped index columns
TOTAL = 4096 * 26 * 2

_CACHE = {}
LAST_RESULTS = None


def _ensure_ntff_hook():
    """Install the axon NTFF profiling hook if missing (test-only path)."""
    import sys
    import types

    if "antenv.axon_hooks" not in sys.modules:
        mod = types.ModuleType("antenv.axon_hooks")
        store = {"hook": None}
        mod.set_axon_ntff_profile_hook = lambda h: store.update(hook=h)
        mod.get_axon_ntff_profile_hook = lambda: store["hook"]
        sys.modules["antenv.axon_hooks"] = mod
        import antenv

        antenv.axon_hooks = mod
    from antenv.axon_hooks import (
        get_axon_ntff_profile_hook,
        set_axon_ntff_profile_hook,
    )

    if get_axon_ntff_profile_hook() is None:
        from trn_agent_boot.trn_boot import _ntff_profile_via_ctypes

        set_axon_ntff_profile_hook(
            _ntff_profile_via_ctypes("/opt/axon/libaxon_pjrt.so")
        )

    import concourse.bass_utils as bu

    bu.upload_artifacts = lambda tmpdir: tmpdir


def _build():
    nc = bacc.Bacc(
        "TRN2", target_bir_lowering=False, debug=False, num_devices=N_CORES
    )
    tab = nc.dram_tensor("tab", [P, CHUNK], mybir.dt.float32, kind="ExternalInput").ap()
    idx = nc.dram_tensor("idx", [P, S], mybir.dt.int16, kind="ExternalInput").ap()
    out = nc.dram_tensor("out", [P, NI], mybir.dt.float32, kind="ExternalOutput").ap()

    with tile.TileContext(nc) as tc:
        with tc.tile_pool(name="sbuf", bufs=1) as pool:
            # Fire the overlay-library load first: its fixed latency is the
            # critical path and overlaps the HWDGE shard/idx loads below.
            nc.gpsimd.load_library(library_config.ap_gather)
            # Tiny dummy gather: walrus drains (= waits for the library
            # load) before each overlay instruction, and that drain runs
            # after the instruction's data waits. The dummy has no DMA
            # deps, so it absorbs the ~200 us load wait at t~=0 while the
            # shard DMA streams; the real gather's drain is then instant.
            dummy_t = pool.tile([P, 64], mybir.dt.float32, tag="dt")
            dummy_i = pool.tile([P, 4], mybir.dt.int16, tag="di")
            dummy_o = pool.tile([P, 64], mybir.dt.float32, tag="do")
            nc.vector.memset(dummy_t[:], 0.0)
            nc.vector.memset(dummy_i[:], 0)
            nc.gpsimd.ap_gather(
                out_ap=dummy_o[:], in_ap=dummy_t[:], idxs_ap=dummy_i[:],
                channels=P, num_elems=64, d=1, num_idxs=64,
            )
            tab_sb = pool.tile([P, CHUNK], mybir.dt.float32, tag="t")
            idx_sb = pool.tile([P, S], mybir.dt.int16, tag="i")
            out_sb = pool.tile([P, NI], mybir.dt.float32, tag="o")
            nc.sync.dma_start(idx_sb[:], idx[:])
            nc.sync.dma_start(tab_sb[:], tab[:])
            nc.gpsimd.ap_gather(
                out_ap=out_sb[:],
                in_ap=tab_sb[:],
                idxs_ap=idx_sb[:],
                channels=P,
                num_elems=CHUNK,
                d=1,
                num_idxs=NI,
            )
            nc.sync.dma_start(out[:], out_sb[:])
    nc.compile()
    return nc


def kernel(inputs: np.ndarray, table: np.ndarray) -> np.ndarray:
    global LAST_RESULTS
    if "nc" not in _CACHE:
        _CACHE["nc"] = _build()
    nc = _CACHE["nc"]

    trace = bool(os.environ.get("BASS_TRACE"))
    if trace:
        _ensure_ntff_hook()

    flat = np.ascontiguousarray(inputs).reshape(-1).astype(np.int64)
    table = np.ascontiguousarray(table, dtype=np.float32)
    assert flat.shape == (TOTAL,) and table.shape == (VOCAB, D)

    owner = flat // SHARD
    local = flat % SHARD
    chunk = local // CHUNK
    pos = (local % CHUNK).astype(np.int16)

    in_maps = []
    placements = []  # per core: list of (orig_positions, n) per chunk
    for o in range(N_CORES):
        tab_np = np.ascontiguousarray(
            table[o * SHARD : (o + 1) * SHARD]
            .reshape(NCH, CHUNK, D)
            .transpose(0, 2, 1)
            .reshape(P, CHUNK)
        )
        idx_groups = np.zeros((8, NI), dtype=np.int16)
        per_chunk = []
        o_mask = owner == o
        for c in range(NCH):
            orig = np.flatnonzero(o_mask & (chunk == c))
            n = len(orig)
            assert n <= NI, f"chunk overflow: {n} > {NI}"
            idx_groups[2 * c, :n] = pos[orig]
            idx_groups[2 * c + 1, :n] = pos[orig]
            per_chunk.append((orig, n))
        # group g's token j lives at idxs[16*g + j%16, j//16]
        idx_np = np.ascontiguousarray(
            idx_groups.reshape(8, S, 16).transpose(0, 2, 1).reshape(P, S)
        )
        in_maps.append({"tab": tab_np, "idx": idx_np})
        placements.append(per_chunk)

    res = run_bass_kernel_spmd(
        nc, in_maps, core_ids=list(range(N_CORES)), trace=trace
    )
    LAST_RESULTS = res

    final = np.empty((TOTAL, D), dtype=np.float32)
    for o in range(N_CORES):
        out = res.results[o]["out"].reshape(P, NI)
        for c in range(NCH):
            orig, n = placements[o][c]
            if n:
                final[orig] = out[32 * c : 32 * (c + 1), :n].T
    return final.reshape(4096, 26, 2, D)
